# revision 32
# baseline (speedup 1.0000x reference)
"""Multi-head attention (B=4, S=2048, D=1024, H=16, DK=64) on 8 TRN2 cores.

Sharding: core c = (b, g) with b = c//2 (data parallel on batch) and g = c%2
(tensor parallel on heads: 8 heads / 512 d' columns per group). Host sums the
two partial output projections per batch and adds bo.

v3 changes vs v2 (trace-driven):
  - Startup DMAs issued in need-order (K-chain, Q-chain, V, xk/xv
    interleaved, weight remainders + wo last) so the first scores start
    DMA-limited rather than queue-order-limited.
  - Global AV deque with depth 2 (4 during the DMA-paced first pair): AV
    matmuls consume exp tiles from >=2 units back so the PE never waits on
    the ACT engine's one-unit lag.
  - Pair finalize goes through SBUF stage tiles ([65,512] f32, one DVE copy
    per head) freeing the PSUM accumulator banks in ~1.3us; softmax
    normalization multiplies read the stage directly (GPS) and write the
    bf16 atn tiles, removing the extra copies.
  - Softmax denominators: reciprocal_approx_fast on the stage rows (f32),
    then DRAM round-trip broadcast per head.  Normalization runs per PAIR,
    one pair behind the units, so every chunk (including the last) hides the
    chain; the old 3.3us DVE reciprocals and the 16us tail stall are gone.
  - Tail: deferred ready out-projections keep the PE warm while the last
    pair's normalization chain resolves; first tail chains pre-run their
    p0..p2 accumulation.
"""

import os
import sys
import time
import types

sys.path.insert(0, "/opt/trn_rl_repo")

import numpy as np
import ml_dtypes


def _install_axon_hooks():
    import antenv

    if "antenv.axon_hooks" in sys.modules:
        return
    hooks = types.ModuleType("antenv.axon_hooks")
    hooks._hook = None
    hooks.set_axon_ntff_profile_hook = lambda h: setattr(hooks, "_hook", h)
    hooks.get_axon_ntff_profile_hook = lambda: hooks._hook
    sys.modules["antenv.axon_hooks"] = hooks
    antenv.axon_hooks = hooks
    try:
        from trn_agent_boot.trn_boot import _ntff_profile_via_ctypes

        hooks.set_axon_ntff_profile_hook(
            _ntff_profile_via_ctypes("/opt/axon/libaxon_pjrt.so")
        )
    except Exception:
        pass


_install_axon_hooks()

import concourse.bacc as bacc
import concourse.bass as bass
import concourse.tile as tile
from concourse import mybir
from concourse import bass_utils
from concourse.bass_utils import run_bass_kernel_spmd

bass_utils.upload_artifacts = lambda tmpdir: tmpdir

BF16 = mybir.dt.bfloat16
F32 = mybir.dt.float32
ALU = mybir.AluOpType

B, S, D = 4, 2048, 1024
H, DK = 16, 64
N_CORES = 8
HC = H // N_CORES * B  # heads per core = 8
DPC = HC * DK  # d' columns per core = 512

LAST_EXEC_TIME_NS = None


def build_program(s=S, dm=D, hc=HC, e=D):
    dk = DK
    dpc = hc * dk
    pairs = hc // 2
    dt_n = dm // 128  # contraction tiles for projections (8)
    st_n = s // 128  # k-tiles (16)
    qc_n = s // 512  # q-chunks (4)
    ec_n = e // 512  # out-proj column chunks (2)

    nc = bacc.Bacc("TRN2", target_bir_lowering=False, debug=False,
                   num_devices=N_CORES)

    xqT = nc.dram_tensor("xqT", [dm, s], BF16, kind="ExternalInput")
    xkT = nc.dram_tensor("xkT", [dm, s], BF16, kind="ExternalInput")
    xvT = nc.dram_tensor("xvT", [dm, s], BF16, kind="ExternalInput")
    wq = nc.dram_tensor("wq", [dm, dpc], BF16, kind="ExternalInput")
    wk = nc.dram_tensor("wk", [dm, dpc], BF16, kind="ExternalInput")
    wv = nc.dram_tensor("wv", [dm, dpc], BF16, kind="ExternalInput")
    wo = nc.dram_tensor("wo", [dpc, e], BF16, kind="ExternalInput")
    bq = nc.dram_tensor("bq", [dpc], F32, kind="ExternalInput")
    bk = nc.dram_tensor("bk", [dpc], F32, kind="ExternalInput")
    out = nc.dram_tensor("out", [s, e], F32, kind="ExternalOutput")

    def step(qc, pr, kk):
        return (qc * pairs + pr) * st_n + kk

    with tile.TileContext(nc) as tc:
        with (
            tc.tile_pool(name="singles", bufs=1) as singles,
            tc.tile_pool(name="xkin", bufs=1) as xkin,
            tc.tile_pool(name="xqin", bufs=2) as xqin,
            tc.tile_pool(name="xvin", bufs=4) as xvin,
            tc.tile_pool(name="expst", bufs=5) as expst_pool,
            tc.tile_pool(name="atn", bufs=hc) as atn_pool,
            tc.tile_pool(name="stage", bufs=4) as stage_pool,
            tc.tile_pool(name="small", bufs=2) as small,
            tc.tile_pool(name="outsb", bufs=2) as outsb_pool,
            tc.tile_pool(name="ps_sc", bufs=2, space="PSUM") as ps_sc,
            tc.tile_pool(name="ps_at", bufs=2, space="PSUM") as ps_at,
            tc.tile_pool(name="ps_mm", bufs=2, space="PSUM") as ps_mm,
            tc.tile_pool(name="dramb", bufs=4, space="DRAM") as dramb,
        ):
            # ---- persistent SBUF tensors ----
            qt_sb = singles.tile([128, pairs, s], BF16, tag="qt")
            kt_sb = singles.tile([128, pairs, s], BF16, tag="kt")
            vn_sb = singles.tile([128, st_n, hc, dk + 1], BF16, tag="vn")
            wq_sb = singles.tile([128, dt_n, dpc], BF16, tag="wq")
            wk_sb = singles.tile([128, dt_n, dpc], BF16, tag="wk")
            wv_sb = singles.tile([128, dt_n, dpc], BF16, tag="wv")
            wo_sb = singles.tile([128, pairs, e], BF16, tag="wo")
            bqc_sb = singles.tile([128, pairs], F32, tag="bqc")
            bkc_sb = singles.tile([128, pairs], F32, tag="bkc")
            ones_sb = singles.tile([128, 512], BF16, tag="ones")
            xk_sb = xkin.tile([128, qc_n, dt_n, 512], BF16, tag="xk")

            wk_src = wk.ap().rearrange("(t p) n -> p t n", p=128)
            wq_src = wq.ap().rearrange("(t p) n -> p t n", p=128)
            xk_src = xkT.ap().rearrange("(t p) n -> p t n", p=128)
            xq_src = xqT.ap().rearrange("(t p) n -> p t n", p=128)
            xv_src = xvT.ap().rearrange("(t p) n -> p t n", p=128)

            # ---- startup DMAs in need-order ----
            # tiny bias vectors first (must not queue behind MB transfers)
            nc.sync.dma_start(
                out=bqc_sb, in_=bq.ap().rearrange("(pr p) -> p pr", p=128))
            nc.sync.dma_start(
                out=bkc_sb, in_=bk.ap().rearrange("(pr p) -> p pr", p=128))
            # wave 1+2 interleaved per d-tile pair: the first KT and QT
            # chains run concurrently, each matmul starting as its (w, x)
            # tiles land -- the chains finish with the DMA instead of after
            xq_blocks = {}
            xq_blocks[0] = xqin.tile([128, dt_n, 512], BF16, tag="xq", name="xqb")
            for t0 in range(0, dt_n, 2):
                ts2 = slice(t0, t0 + 2)
                nc.sync.dma_start(
                    out=wk_sb[:, ts2, 0:128], in_=wk_src[:, ts2, 0:128])
                nc.sync.dma_start(
                    out=xk_sb[:, 0, ts2, :], in_=xk_src[:, ts2, 0:512])
                nc.sync.dma_start(
                    out=wq_sb[:, ts2, 0:128], in_=wq_src[:, ts2, 0:128])
                nc.sync.dma_start(
                    out=xq_blocks[0][:, ts2, :], in_=xq_src[:, ts2, 0:512])
            # wave 3: V path + remaining xk, interleaved by need time
            wv_src = wv.ap().rearrange("(t p) n -> p t n", p=128)
            nc.sync.dma_start(out=wv_sb[:, 0:4, :], in_=wv_src[:, 0:4, :])
            nc.sync.dma_start(out=wv_sb[:, 4:8, :], in_=wv_src[:, 4:8, :])

            xv_blocks = {}

            def issue_xv(nj):
                blk = xvin.tile([128, dt_n, 256], BF16, tag="xv", name="xvb")
                nc.sync.dma_start(
                    out=blk, in_=xv_src[:, :, nj * 256:(nj + 1) * 256])
                xv_blocks[nj] = blk

            def ensure_xv(j):
                while len(xv_blocks) <= min(j + 2, s // 256 - 1):
                    issue_xv(len(xv_blocks))

            issue_xv(0)
            nc.sync.dma_start(
                out=xk_sb[:, 1, :, :], in_=xk_src[:, :, 512:1024])
            issue_xv(1)
            nc.sync.dma_start(
                out=xk_sb[:, 2, :, :], in_=xk_src[:, :, 1024:1536])
            issue_xv(2)
            issue_xv(3)
            nc.sync.dma_start(
                out=xk_sb[:, 3, :, :], in_=xk_src[:, :, 1536:2048])
            # xv blocks 4..7 pre-issued: their triggers wait on the ring
            # semaphores (earlier blocks consumed) and release in need order,
            # ahead of the weight remainders in the DMA FIFO.
            issue_xv(4)
            issue_xv(5)
            issue_xv(6)
            issue_xv(7)
            # wave 4: weight remainders (needed from pair 1 on), wo last
            nc.sync.dma_start(
                out=wk_sb[:, :, 128:dpc], in_=wk_src[:, :, 128:dpc])
            nc.sync.dma_start(
                out=wq_sb[:, :, 128:dpc], in_=wq_src[:, :, 128:dpc])
            nc.sync.dma_start(
                out=wo_sb, in_=wo.ap().rearrange("(a p) e -> p a e", p=128))
            # pair-3 head-B rows of wo at base partition 0, for the tail's
            # per-head p3 matmuls (matmul needs equal base partitions)
            wo_b3 = singles.tile([64, e], BF16, tag="wob3")
            nc.sync.dma_start(out=wo_b3, in_=wo.ap()[dpc - 64:dpc, :])

            nc.vector.memset(ones_sb, 1.0)
            nc.vector.memset(vn_sb[:, :, :, dk:dk + 1], 1.0)

            # Warm-up ACT: the Ln+Exp pair narrows the activation-table
            # choice to the table containing BOTH, so the tail's ln/exp
            # reciprocal needs no further table loads.
            warm_sb = singles.tile([128, 32], F32, tag="warm")
            nc.scalar.activation(
                warm_sb, ones_sb[:, 0:32], mybir.ActivationFunctionType.Ln)
            nc.scalar.activation(
                warm_sb, ones_sb[:, 0:32], mybir.ActivationFunctionType.Exp)

            # PE p-state warm-up: junk matmuls during the startup DMA window
            # ramp the PE clock (0.65 -> 2.4 GHz needs ~3us of continuous
            # execution) so the first real chains run at full speed.
            jnk_ps = ps_mm.tile([128, 512], F32, tag="ps")
            for _ in range(50):
                nc.tensor.matmul(
                    jnk_ps[:, 0:128], ones_sb[:, 0:128], ones_sb[:, 0:128],
                    start=True, stop=True)

            # ---- helper emitters ----
            def qk_chain(w_sb, b_sb, xs, dst, p, qcc):
                """QT/KT chain: 8 matmuls + biased eviction (no bias mm)."""
                ps = ps_mm.tile([128, 512], F32, tag="ps")
                for t in range(dt_n):
                    nc.tensor.matmul(
                        ps,
                        w_sb[:, t, p * 128:(p + 1) * 128],
                        xs[:, t, :],
                        start=(t == 0),
                        stop=(t == dt_n - 1),
                    )
                nc.vector.tensor_scalar(
                    dst[:, p, qcc * 512:(qcc + 1) * 512], ps,
                    b_sb[:, p:p + 1], None, ALU.add)

            def qk_gen(w_sb, b_sb, xs, dst, p, qcc):
                ps = ps_mm.tile([128, 512], F32, tag="ps")
                for t in range(dt_n):
                    nc.tensor.matmul(
                        ps,
                        w_sb[:, t, p * 128:(p + 1) * 128],
                        xs[:, t, :],
                        start=(t == 0),
                        stop=(t == dt_n - 1),
                    )
                    yield
                nc.vector.tensor_scalar(
                    dst[:, p, qcc * 512:(qcc + 1) * 512], ps,
                    b_sb[:, p:p + 1], None, ALU.add)
                yield

            def v_gen(st):
                """V chain for k-tile st: 8 matmuls + evict (ones col = 1)."""
                ensure_xv(st // 2)
                blk = xv_blocks[st // 2]
                off = (st % 2) * 128
                ps = ps_mm.tile([128, 512], F32, tag="ps")
                for t in range(dt_n):
                    nc.tensor.matmul(
                        ps,
                        blk[:, t, off:off + 128],
                        wv_sb[:, t, :],
                        start=(t == 0),
                        stop=(t == dt_n - 1),
                    )
                    yield
                nc.vector.tensor_copy(
                    vn_sb[:, st, :, 0:dk],
                    ps.rearrange("p (h d) -> p h d", d=dk),
                )
                yield

            atn_tiles = {}  # (qc, pr) -> atn_pair tile (normalized, bf16)
            norm_stash = {}  # (qc, pr) -> (stage_A, stage_B, rec_dram)
            prev_pair = [None]  # last finalized (qc, pr) not yet normalized

            def emit_norm(qc, pr):
                """Normalize pair (qc, pr): broadcast 1/den and multiply the
                f32 stage into the bf16 atn tile (head B via DMA shift)."""
                stage_A, stage_B, rd = norm_stash.pop((qc, pr))
                atn_pair = atn_pool.tile([128, 512], BF16, tag="atn")
                for h, st_t in ((0, stage_A), (1, stage_B)):
                    bc = small.tile([64, 512], F32, tag="bc")
                    row = rd[h:h + 1, :]
                    bcast_src = bass.AP(
                        tensor=row.tensor,
                        offset=row.offset,
                        ap=[[0, 64]] + list(row.ap[1:]),
                    )
                    nc.sync.dma_start(out=bc, in_=bcast_src)
                    if h == 0:
                        nc.gpsimd.tensor_mul(
                            atn_pair[0:64, :], st_t[0:64, :], bc)
                    else:
                        btmp = small.tile([64, 512], BF16, tag="btmp")
                        nc.gpsimd.tensor_mul(btmp, st_t[0:64, :], bc)
                        nc.sync.dma_start(out=atn_pair[64:128, :], in_=btmp)
                atn_tiles[(qc, pr)] = atn_pair

            def outproj_gen(qcc, qt_i, ecc):
                esl = slice(ecc * 512, (ecc + 1) * 512)
                q0 = qcc * 4 + qt_i
                o_ps = ps_mm.tile([128, 512], F32, tag="ps")
                for p in range(pairs):
                    nc.tensor.matmul(
                        o_ps,
                        atn_tiles[(qcc, p)][:, qt_i * 128:(qt_i + 1) * 128],
                        wo_sb[:, p, esl],
                        start=(p == 0),
                        stop=(p == pairs - 1),
                    )
                    yield
                o_sb = outsb_pool.tile([128, 512], F32, tag="o")
                nc.vector.tensor_copy(o_sb, o_ps)
                nc.sync.dma_start(
                    out=out.ap()[q0 * 128:(q0 + 1) * 128, esl], in_=o_sb)
                yield

            class FillerQueue:
                def __init__(self):
                    self.tasks = []  # (gen, deadline_step or None)

                def add(self, gen, deadline=None):
                    self.tasks.append((gen, deadline))

                def pump(self, n):
                    while n > 0 and self.tasks:
                        try:
                            next(self.tasks[0][0])
                            n -= 1
                        except StopIteration:
                            self.tasks.pop(0)

                def fence(self, cur):
                    while self.tasks and any(
                        dl is not None and dl <= cur for _, dl in self.tasks
                    ):
                        self.pump(1)

                def drain(self):
                    while self.tasks:
                        self.pump(1000)

            fill = FillerQueue()

            # ---- stage A: just enough for the first scores ----
            # KT and QT chains interleaved per d-tile, paced by the
            # interleaved startup DMAs (two concurrent PSUM accumulators)
            psK = ps_mm.tile([128, 512], F32, tag="ps")
            psQ = ps_mm.tile([128, 512], F32, tag="ps")
            for t in range(dt_n):
                nc.tensor.matmul(
                    psK, wk_sb[:, t, 0:128], xk_sb[:, 0, t, :],
                    start=(t == 0), stop=(t == dt_n - 1))
                nc.tensor.matmul(
                    psQ, wq_sb[:, t, 0:128], xq_blocks[0][:, t, :],
                    start=(t == 0), stop=(t == dt_n - 1))
            nc.vector.tensor_scalar(
                kt_sb[:, 0, 0:512], psK, bkc_sb[:, 0:1], None, ALU.add)
            nc.vector.tensor_scalar(
                qt_sb[:, 0, 0:512], psQ, bqc_sb[:, 0:1], None, ALU.add)

            # stage-A remainder as deadline fillers (all within qc0).
            # AV(kk) is emitted at unit kk+4 during the first pair, so V(st)
            # deadlines get a 2-unit margin past st.
            def vdl(st):
                return step(0, 0, min(st + 2, st_n - 1))

            fill.add(v_gen(0), deadline=step(0, 0, 1))
            fill.add(v_gen(1), deadline=step(0, 0, 2))
            fill.add(v_gen(2), deadline=vdl(2))
            for c in range(1, qc_n):
                fill.add(qk_gen(wk_sb, bkc_sb, xk_sb[:, c, :, :], kt_sb, 0, c),
                         deadline=step(0, 0, 4 * c - 1))
                fill.add(v_gen(2 * c + 1), deadline=vdl(2 * c + 1))
                fill.add(v_gen(2 * c + 2), deadline=vdl(2 * c + 2))
            for st in range(9, st_n):
                fill.add(v_gen(st), deadline=vdl(st))
            # later pairs' KT/QT (kt/qt needed when unit (0, p) starts)
            for p in range(1, pairs):
                fill.add(qk_gen(wq_sb, bqc_sb, xq_blocks[0], qt_sb, p, 0),
                         deadline=step(0, p, 0))
                for c in range(qc_n):
                    fill.add(
                        qk_gen(wk_sb, bkc_sb, xk_sb[:, c, :, :], kt_sb, p, c),
                        deadline=step(0, p, max(4 * c - 1, 0)))

            # ---- AV deque: entries emit the AV matmuls for one unit ----
            avq = []  # (emit_fn, finalize_fn or None)

            def drain_avq(target):
                while len(avq) > target:
                    em, fin = avq.pop(0)
                    em()
                    if fin is not None:
                        fin()

            # ---- main loop over q-chunks and pairs ----
            for qc in range(qc_n):
                qsl = slice(qc * 512, (qc + 1) * 512)
                last = qc == qc_n - 1

                # prefetch xq block qc+1 and enqueue its QT chains
                if qc + 1 < qc_n:
                    blkq = xqin.tile([128, dt_n, 512], BF16, tag="xq")
                    nc.sync.dma_start(
                        out=blkq,
                        in_=xq_src[:, :, (qc + 1) * 512:(qc + 2) * 512])
                    xq_blocks[qc + 1] = blkq
                    for pp in range(pairs):
                        # spread through this chunk's pair 0 (which has ACT
                        # slack) instead of piling into pair 1
                        dl = (step(qc, 0, 7 + 2 * pp) if qc > 0
                              else step(qc + 1, pp, 0))
                        fill.add(
                            qk_gen(wq_sb, bqc_sb, blkq, qt_sb, pp, qc + 1),
                            deadline=dl)

                for pr in range(pairs):
                    at_A = ps_at.tile([65, 512], F32, tag="at")
                    at_B = ps_at.tile([65, 512], F32, tag="at")

                    def make_emit(at_A, at_B, pr, kk, e_sb):
                        def em():
                            nc.tensor.matmul(
                                at_A,
                                vn_sb[:, kk, 2 * pr, :],
                                e_sb[:, 0:512],
                                start=(kk == 0),
                                stop=(kk == st_n - 1),
                            )
                            nc.tensor.matmul(
                                at_B,
                                vn_sb[:, kk, 2 * pr + 1, :],
                                e_sb[:, 512:1024],
                                start=(kk == 0),
                                stop=(kk == st_n - 1),
                            )
                        return em

                    def make_finalize(at_A, at_B, qc, pr):
                        def fin():
                            if (qc, pr) == (qc_n - 1, pairs - 1):
                                # last pair: the tail normalizes straight
                                # from the PSUM accumulators (no stage
                                # copies; the banks die with the program)
                                norm_stash[(qc, pr)] = (at_A, at_B, None)
                                if prev_pair[0] is not None:
                                    emit_norm(*prev_pair[0])
                                prev_pair[0] = (qc, pr)
                                return
                            # stage the accumulators (frees both PSUM banks
                            # after one DVE copy each)
                            stage_A = stage_pool.tile([65, 512], F32, tag="st")
                            stage_B = stage_pool.tile([65, 512], F32, tag="st")
                            nc.vector.tensor_copy(stage_A, at_A)
                            nc.vector.tensor_copy(stage_B, at_B)
                            if True:
                                # scatter the [1,512] denominator rows across
                                # 128 partitions so the DVE reciprocal runs on
                                # a free-size-4 AP (~0.15us instead of 3.3us),
                                # then gather back to DRAM for the bcast
                                rsc = small.tile([128, 8], F32, tag="rsc")
                                nc.sync.dma_start(
                                    out=rsc[:, 0:4], in_=stage_A[64:65, :])
                                nc.sync.dma_start(
                                    out=rsc[:, 4:8], in_=stage_B[64:65, :])
                                rcp = small.tile([128, 8], F32, tag="rcp")
                                nc.vector.reciprocal(out=rcp, in_=rsc)
                                rd = dramb.tile([2, 512], F32, tag="rd")
                                nc.sync.dma_start(
                                    out=rd[0:1, :], in_=rcp[:, 0:4])
                                nc.sync.dma_start(
                                    out=rd[1:2, :], in_=rcp[:, 4:8])
                                norm_stash[(qc, pr)] = (stage_A, stage_B, rd)
                            # normalize the previous pair (its rec chain has
                            # had a full pair of lead time)
                            if prev_pair[0] is not None:
                                emit_norm(*prev_pair[0])
                                # when crossing into chunk qc, the previous
                                # chunk becomes fully normalized -> queue its
                                # out-projections
                                pq, ppr = prev_pair[0]
                                if ppr == pairs - 1:
                                    # first deadline leaves time for the
                                    # norm chain of (pq, 3) to resolve
                                    dls = [(1, 13), (1, 15), (2, 3), (2, 7),
                                           (2, 11), (2, 15), (3, 3), (3, 7)]
                                    for sq in range(4 * ec_n):
                                        fill.add(
                                            outproj_gen(
                                                pq, sq // ec_n, sq % ec_n),
                                            deadline=step(pq + 1, *dls[sq]))
                            prev_pair[0] = (qc, pr)
                        return fin

                    for kk in range(st_n):
                        fill.fence(step(qc, pr, kk))
                        sc_ps = ps_sc.tile([128, 1024], F32, tag="sc")
                        ksl = slice(kk * 128, (kk + 1) * 128)
                        nc.tensor.matmul(
                            sc_ps[:, 0:512],
                            kt_sb[0:64, pr, ksl],
                            qt_sb[0:64, pr, qsl],
                            start=True,
                            stop=True,
                        )
                        nc.tensor.matmul(
                            sc_ps[:, 512:1024],
                            kt_sb[64:128, pr, ksl],
                            qt_sb[64:128, pr, qsl],
                            start=True,
                            stop=True,
                        )
                        exp_sb = expst_pool.tile([128, 1024], BF16, tag="e")
                        nc.scalar.activation(
                            exp_sb, sc_ps,
                            mybir.ActivationFunctionType.Exp,
                            scale=1.0 / np.sqrt(dk),
                        )
                        fin = (make_finalize(at_A, at_B, qc, pr)
                               if kk == st_n - 1 else None)
                        avq.append(
                            (make_emit(at_A, at_B, pr, kk, exp_sb), fin))
                        if qc == 0 and pr == 0:
                            depth = 4
                        elif qc == 0 and pr == 1:
                            depth = 3
                        else:
                            depth = 2
                        drain_avq(depth)
                        fill.pump(2)

            # ---- tail ----
            fill.drain()
            drain_avq(0)  # AV(14), AV(15) + finalize of the last pair
            # Last-pair normalization, no DMA hops: DVE reciprocals straight
            # off the PSUM denominator rows (emitted first so they lead the
            # DVE queue), then -- AFTER the warm reserve matmuls are queued
            # on the PE -- an outer-product broadcast into free ps_sc banks
            # and DVE multiplies into per-head tiles.  The p3 matmuls are
            # split per head, so no partition-shift DMA anywhere.
            lq = qc_n - 1
            at_lA, at_lB, _ = norm_stash.pop((lq, pairs - 1))
            # 1/den = exp(-ln(den)) on the (now idle) ACT engine -- Ln and
            # Exp share an activation table, so this costs ~1.5us with no
            # DVE reciprocal and no DMA hops.
            ln_A = small.tile([1, 512], F32, tag="rbf")
            ln_B = small.tile([1, 512], F32, tag="rbf")
            nc.scalar.activation(
                ln_A, at_lA[64:65, :], mybir.ActivationFunctionType.Ln)
            nc.scalar.activation(
                ln_B, at_lB[64:65, :], mybir.ActivationFunctionType.Ln)
            rec_bA = small.tile([1, 512], BF16, tag="rbb")
            rec_bB = small.tile([1, 512], BF16, tag="rbb")
            nc.scalar.activation(
                rec_bA, ln_A, mybir.ActivationFunctionType.Exp, scale=-1.0)
            nc.scalar.activation(
                rec_bB, ln_B, mybir.ActivationFunctionType.Exp, scale=-1.0)
            # SBUF copies of the accumulators (the multiply may read only
            # one PSUM operand -- the bcast stays in PSUM)
            st_lA = stage_pool.tile([65, 512], F32, tag="st")
            st_lB = stage_pool.tile([65, 512], F32, tag="st")
            nc.vector.tensor_copy(st_lA[0:64, :], at_lA[0:64, :])
            nc.vector.tensor_copy(st_lB[0:64, :], at_lB[0:64, :])

            bc_ps = ps_sc.tile([128, 1024], F32, tag="sc")
            nc.tensor.matmul(
                bc_ps[0:64, 0:512], ones_sb[0:1, 0:64], rec_bA,
                start=True, stop=True)
            nc.tensor.matmul(
                bc_ps[0:64, 512:1024], ones_sb[0:1, 0:64], rec_bB,
                start=True, stop=True)
            atnA = small.tile([64, 512], BF16, tag="tA")
            atnB = small.tile([64, 512], BF16, tag="tB")
            nc.vector.tensor_mul(atnA, st_lA[0:64, :], bc_ps[0:64, 0:512])
            nc.vector.tensor_mul(atnB, st_lB[0:64, :], bc_ps[0:64, 512:1024])

            def tail_chain(o_ps, qt_i, esl, start):
                """p0..p2 accumulation for a last-chunk out-proj chain."""
                for p in range(pairs - 1):
                    nc.tensor.matmul(
                        o_ps,
                        atn_tiles[(lq, p)][:, qt_i * 128:(qt_i + 1) * 128],
                        wo_sb[:, p, esl],
                        start=(p == 0 and start),
                        stop=False,
                    )

            def tail_p3(o_ps, qt_i, ecc, esl):
                """Per-head p3 matmuls + eviction for a last-chunk chain."""
                qsl2 = slice(qt_i * 128, (qt_i + 1) * 128)
                nc.tensor.matmul(
                    o_ps, atnA[:, qsl2], wo_sb[0:64, pairs - 1, esl],
                    start=False, stop=False)
                nc.tensor.matmul(
                    o_ps, atnB[:, qsl2], wo_b3[:, esl],
                    start=False, stop=True)
                q0 = lq * 4 + qt_i
                o_sb = outsb_pool.tile([128, 512], F32, tag="o")
                nc.vector.tensor_copy(o_sb, o_ps)
                nc.sync.dma_start(
                    out=out.ap()[q0 * 128:(q0 + 1) * 128, esl], in_=o_sb)

            held = []
            for sq in range(2):
                qt_i, ecc = sq // ec_n, sq % ec_n
                esl = slice(ecc * 512, (ecc + 1) * 512)
                o_ps = ps_mm.tile([128, 512], F32, tag="ps")
                tail_chain(o_ps, qt_i, esl, start=True)
                held.append((o_ps, qt_i, ecc, esl))
            for o_ps, qt_i, ecc, esl in held:
                tail_p3(o_ps, qt_i, ecc, esl)
            for sq in range(2, 4 * ec_n):
                qt_i, ecc = sq // ec_n, sq % ec_n
                esl = slice(ecc * 512, (ecc + 1) * 512)
                o_ps = ps_mm.tile([128, 512], F32, tag="ps")
                tail_chain(o_ps, qt_i, esl, start=True)
                tail_p3(o_ps, qt_i, ecc, esl)

    nc.compile()
    return nc


_PROGRAM_CACHE = {}


def _get_program(key):
    if key not in _PROGRAM_CACHE:
        _PROGRAM_CACHE[key] = build_program(*key)
    return _PROGRAM_CACHE[key]


def kernel(queries, keys, values, Wq, bq, Wk, bk, Wv, bv, Wo, bo):
    global LAST_EXEC_TIME_NS
    bf16 = ml_dtypes.bfloat16

    nc = _get_program((S, D, HC, D))

    xT = {}
    for name, arr in (("q", queries), ("k", keys), ("v", values)):
        xT[name] = [
            np.ascontiguousarray(np.asarray(arr[b]).T).astype(bf16)
            for b in range(B)
        ]
    Wq, Wk, Wv, Wo = (np.asarray(w) for w in (Wq, Wk, Wv, Wo))
    bqv, bkv, bvv = (np.asarray(v) for v in (bq, bk, bv))

    in_maps = []
    for c in range(N_CORES):
        b, g = c // 2, c % 2
        csl = slice(g * DPC, (g + 1) * DPC)
        in_maps.append(
            {
                "xqT": xT["q"][b],
                "xkT": xT["k"][b],
                "xvT": xT["v"][b],
                "wq": np.ascontiguousarray(Wq[:, csl]).astype(bf16),
                "wk": np.ascontiguousarray(Wk[:, csl]).astype(bf16),
                "wv": np.ascontiguousarray(Wv[:, csl]).astype(bf16),
                "wo": np.ascontiguousarray(Wo[csl, :]).astype(bf16),
                "bq": np.ascontiguousarray(bqv[csl]).astype(np.float32),
                "bk": np.ascontiguousarray(bkv[csl]).astype(np.float32),
            }
        )

    trace = os.environ.get("KERNEL_TRACE", "0") == "1"
    res = run_bass_kernel_spmd(nc, in_maps, list(range(N_CORES)), trace=trace)
    LAST_EXEC_TIME_NS = res.exec_time_ns

    # bv's contribution commutes through softmax-normalized attention:
    # each head's output gains +bv_h, so the final output gains bv @ Wo.
    bo = np.asarray(bo, dtype=np.float32) + bvv.astype(np.float32) @ Wo.astype(np.float32)
    out = np.empty((B, S, D), dtype=np.float32)
    for b in range(B):
        out[b] = res.results[2 * b]["out"] + res.results[2 * b + 1]["out"] + bo
    return out


if __name__ == "__main__":
    t0 = time.time()
    nc = _get_program((S, D, HC, D))
    print(f"build+compile: {time.time() - t0:.1f}s")


# revision 35
# speedup vs baseline: 1.0356x; 1.0356x over previous
"""Multi-head attention (B=4, S=2048, D=1024, H=16, DK=64) on 8 TRN2 cores.

Sharding: core c = (b, g) with b = c//2 (data parallel on batch) and g = c%2
(tensor parallel on heads: 8 heads / 512 d' columns per group). Host sums the
two partial output projections per batch and adds bo.

v3 changes vs v2 (trace-driven):
  - Startup DMAs issued in need-order (K-chain, Q-chain, V, xk/xv
    interleaved, weight remainders + wo last) so the first scores start
    DMA-limited rather than queue-order-limited.
  - Global AV deque with depth 2 (4 during the DMA-paced first pair): AV
    matmuls consume exp tiles from >=2 units back so the PE never waits on
    the ACT engine's one-unit lag.
  - Pair finalize goes through SBUF stage tiles ([65,512] f32, one DVE copy
    per head) freeing the PSUM accumulator banks in ~1.3us; softmax
    normalization multiplies read the stage directly (GPS) and write the
    bf16 atn tiles, removing the extra copies.
  - Softmax denominators: reciprocal_approx_fast on the stage rows (f32),
    then DRAM round-trip broadcast per head.  Normalization runs per PAIR,
    one pair behind the units, so every chunk (including the last) hides the
    chain; the old 3.3us DVE reciprocals and the 16us tail stall are gone.
  - Tail: deferred ready out-projections keep the PE warm while the last
    pair's normalization chain resolves; first tail chains pre-run their
    p0..p2 accumulation.
"""

import os
import sys
import time
import types

sys.path.insert(0, "/opt/trn_rl_repo")

import numpy as np
import ml_dtypes


def _install_axon_hooks():
    import antenv

    if "antenv.axon_hooks" in sys.modules:
        return
    hooks = types.ModuleType("antenv.axon_hooks")
    hooks._hook = None
    hooks.set_axon_ntff_profile_hook = lambda h: setattr(hooks, "_hook", h)
    hooks.get_axon_ntff_profile_hook = lambda: hooks._hook
    sys.modules["antenv.axon_hooks"] = hooks
    antenv.axon_hooks = hooks
    try:
        from trn_agent_boot.trn_boot import _ntff_profile_via_ctypes

        hooks.set_axon_ntff_profile_hook(
            _ntff_profile_via_ctypes("/opt/axon/libaxon_pjrt.so")
        )
    except Exception:
        pass


_install_axon_hooks()

import concourse.bacc as bacc
import concourse.bass as bass
import concourse.tile as tile
from concourse import mybir
from concourse import bass_utils
from concourse.bass_utils import run_bass_kernel_spmd

bass_utils.upload_artifacts = lambda tmpdir: tmpdir

BF16 = mybir.dt.bfloat16
F32 = mybir.dt.float32
ALU = mybir.AluOpType

B, S, D = 4, 2048, 1024
H, DK = 16, 64
N_CORES = 8
HC = H // N_CORES * B  # heads per core = 8
DPC = HC * DK  # d' columns per core = 512

LAST_EXEC_TIME_NS = None


def build_program(s=S, dm=D, hc=HC, e=D):
    dk = DK
    dpc = hc * dk
    pairs = hc // 2
    dt_n = dm // 128  # contraction tiles for projections (8)
    st_n = s // 128  # k-tiles (16)
    qc_n = s // 512  # q-chunks (4)
    ec_n = e // 512  # out-proj column chunks (2)

    nc = bacc.Bacc("TRN2", target_bir_lowering=False, debug=False,
                   num_devices=N_CORES)

    xqT = nc.dram_tensor("xqT", [dm, s], BF16, kind="ExternalInput")
    xkT = nc.dram_tensor("xkT", [dm, s], BF16, kind="ExternalInput")
    xvT = nc.dram_tensor("xvT", [dm, s], BF16, kind="ExternalInput")
    wq = nc.dram_tensor("wq", [dm, dpc], BF16, kind="ExternalInput")
    wk = nc.dram_tensor("wk", [dm, dpc], BF16, kind="ExternalInput")
    wv = nc.dram_tensor("wv", [dm, dpc], BF16, kind="ExternalInput")
    wo = nc.dram_tensor("wo", [dpc, e], BF16, kind="ExternalInput")
    bq = nc.dram_tensor("bq", [dpc], F32, kind="ExternalInput")
    bk = nc.dram_tensor("bk", [dpc], F32, kind="ExternalInput")
    out = nc.dram_tensor("out", [s, e], F32, kind="ExternalOutput")

    def step(qc, pr, kk):
        return (qc * pairs + pr) * st_n + kk

    with tile.TileContext(nc) as tc:
        with (
            tc.tile_pool(name="singles", bufs=1) as singles,
            tc.tile_pool(name="xkin", bufs=1) as xkin,
            tc.tile_pool(name="xqin", bufs=2) as xqin,
            tc.tile_pool(name="xvin", bufs=4) as xvin,
            tc.tile_pool(name="expst", bufs=5) as expst_pool,
            tc.tile_pool(name="atn", bufs=hc) as atn_pool,
            tc.tile_pool(name="stage", bufs=4) as stage_pool,
            tc.tile_pool(name="small", bufs=2) as small,
            tc.tile_pool(name="outsb", bufs=2) as outsb_pool,
            tc.tile_pool(name="ps_sc", bufs=2, space="PSUM") as ps_sc,
            tc.tile_pool(name="ps_at", bufs=2, space="PSUM") as ps_at,
            tc.tile_pool(name="ps_mm", bufs=2, space="PSUM") as ps_mm,
            tc.tile_pool(name="dramb", bufs=4, space="DRAM") as dramb,
        ):
            # ---- persistent SBUF tensors ----
            qt_sb = singles.tile([128, pairs, s], BF16, tag="qt")
            kt_sb = singles.tile([128, pairs, s], BF16, tag="kt")
            vn_sb = singles.tile([128, st_n, hc, dk + 1], BF16, tag="vn")
            wq_sb = singles.tile([128, dt_n, dpc], BF16, tag="wq")
            wk_sb = singles.tile([128, dt_n, dpc], BF16, tag="wk")
            wv_sb = singles.tile([128, dt_n, dpc], BF16, tag="wv")
            wo_sb = singles.tile([128, pairs, e], BF16, tag="wo")
            bqc_sb = singles.tile([128, pairs], F32, tag="bqc")
            bkc_sb = singles.tile([128, pairs], F32, tag="bkc")
            ones_sb = singles.tile([128, 512], BF16, tag="ones")
            xk_sb = xkin.tile([128, qc_n, dt_n, 512], BF16, tag="xk")

            wk_src = wk.ap().rearrange("(t p) n -> p t n", p=128)
            wq_src = wq.ap().rearrange("(t p) n -> p t n", p=128)
            xk_src = xkT.ap().rearrange("(t p) n -> p t n", p=128)
            xq_src = xqT.ap().rearrange("(t p) n -> p t n", p=128)
            xv_src = xvT.ap().rearrange("(t p) n -> p t n", p=128)

            # ---- startup DMAs in need-order ----
            # tiny bias vectors first (must not queue behind MB transfers)
            nc.sync.dma_start(
                out=bqc_sb, in_=bq.ap().rearrange("(pr p) -> p pr", p=128))
            nc.sync.dma_start(
                out=bkc_sb, in_=bk.ap().rearrange("(pr p) -> p pr", p=128))
            # wave 1+2: big transfers (each dma_start trigger costs ~0.6us
            # serially on the sync queue -- fine splits lose), halves so the
            # interleaved first chains can start on the first half
            xq_blocks = {}
            xq_blocks[0] = xqin.tile([128, dt_n, 512], BF16, tag="xq", name="xqb")
            nc.sync.dma_start(out=wk_sb[:, :, 0:128], in_=wk_src[:, :, 0:128])
            nc.sync.dma_start(out=xk_sb[:, 0, 0:4, :], in_=xk_src[:, 0:4, 0:512])
            nc.sync.dma_start(out=wq_sb[:, :, 0:128], in_=wq_src[:, :, 0:128])
            nc.sync.dma_start(out=xq_blocks[0][:, 0:4, :], in_=xq_src[:, 0:4, 0:512])
            nc.sync.dma_start(out=xk_sb[:, 0, 4:8, :], in_=xk_src[:, 4:8, 0:512])
            nc.sync.dma_start(out=xq_blocks[0][:, 4:8, :], in_=xq_src[:, 4:8, 0:512])
            # wave 3: V path + remaining xk, interleaved by need time
            wv_src = wv.ap().rearrange("(t p) n -> p t n", p=128)
            nc.sync.dma_start(out=wv_sb[:, 0:4, :], in_=wv_src[:, 0:4, :])
            nc.sync.dma_start(out=wv_sb[:, 4:8, :], in_=wv_src[:, 4:8, :])

            xv_blocks = {}

            def issue_xv(nj):
                blk = xvin.tile([128, dt_n, 256], BF16, tag="xv", name="xvb")
                nc.sync.dma_start(
                    out=blk, in_=xv_src[:, :, nj * 256:(nj + 1) * 256])
                xv_blocks[nj] = blk

            def ensure_xv(j):
                while len(xv_blocks) <= min(j + 2, s // 256 - 1):
                    issue_xv(len(xv_blocks))

            issue_xv(0)
            nc.sync.dma_start(
                out=xk_sb[:, 1, :, :], in_=xk_src[:, :, 512:1024])
            issue_xv(1)
            nc.sync.dma_start(
                out=xk_sb[:, 2, :, :], in_=xk_src[:, :, 1024:1536])
            issue_xv(2)
            issue_xv(3)
            nc.sync.dma_start(
                out=xk_sb[:, 3, :, :], in_=xk_src[:, :, 1536:2048])
            # xv blocks 4..7 pre-issued: their triggers wait on the ring
            # semaphores (earlier blocks consumed) and release in need order,
            # ahead of the weight remainders in the DMA FIFO.
            issue_xv(4)
            issue_xv(5)
            issue_xv(6)
            issue_xv(7)
            # wave 4: weight remainders (needed from pair 1 on), wo last
            nc.sync.dma_start(
                out=wk_sb[:, :, 128:dpc], in_=wk_src[:, :, 128:dpc])
            nc.sync.dma_start(
                out=wq_sb[:, :, 128:dpc], in_=wq_src[:, :, 128:dpc])
            nc.sync.dma_start(
                out=wo_sb, in_=wo.ap().rearrange("(a p) e -> p a e", p=128))
            # pair-3 head-B rows of wo at base partition 0, for the tail's
            # per-head p3 matmuls (matmul needs equal base partitions)
            wo_b3 = singles.tile([64, e], BF16, tag="wob3")
            nc.sync.dma_start(out=wo_b3, in_=wo.ap()[dpc - 64:dpc, :])

            nc.vector.memset(ones_sb, 1.0)
            nc.vector.memset(vn_sb[:, :, :, dk:dk + 1], 1.0)

            # Warm-up ACT: the Ln+Exp pair narrows the activation-table
            # choice to the table containing BOTH, so the tail's ln/exp
            # reciprocal needs no further table loads.
            warm_sb = singles.tile([128, 32], F32, tag="warm")
            nc.scalar.activation(
                warm_sb, ones_sb[:, 0:32], mybir.ActivationFunctionType.Ln)
            nc.scalar.activation(
                warm_sb, ones_sb[:, 0:32], mybir.ActivationFunctionType.Exp)

            # PE p-state warm-up: junk matmuls during the startup DMA window
            # ramp the PE clock (0.65 -> 2.4 GHz needs ~3us of continuous
            # execution) so the first real chains run at full speed.
            jnk_ps = ps_mm.tile([128, 512], F32, tag="ps")
            for _ in range(50):
                nc.tensor.matmul(
                    jnk_ps[:, 0:128], ones_sb[:, 0:128], ones_sb[:, 0:128],
                    start=True, stop=True)

            # ---- helper emitters ----
            def qk_chain(w_sb, b_sb, xs, dst, p, qcc):
                """QT/KT chain: 8 matmuls + biased eviction (no bias mm)."""
                ps = ps_mm.tile([128, 512], F32, tag="ps")
                for t in range(dt_n):
                    nc.tensor.matmul(
                        ps,
                        w_sb[:, t, p * 128:(p + 1) * 128],
                        xs[:, t, :],
                        start=(t == 0),
                        stop=(t == dt_n - 1),
                    )
                nc.vector.tensor_scalar(
                    dst[:, p, qcc * 512:(qcc + 1) * 512], ps,
                    b_sb[:, p:p + 1], None, ALU.add)

            def qk_gen(w_sb, b_sb, xs, dst, p, qcc):
                ps = ps_mm.tile([128, 512], F32, tag="ps")
                for t in range(dt_n):
                    nc.tensor.matmul(
                        ps,
                        w_sb[:, t, p * 128:(p + 1) * 128],
                        xs[:, t, :],
                        start=(t == 0),
                        stop=(t == dt_n - 1),
                    )
                    yield
                nc.vector.tensor_scalar(
                    dst[:, p, qcc * 512:(qcc + 1) * 512], ps,
                    b_sb[:, p:p + 1], None, ALU.add)
                yield

            def v_gen(st):
                """V chain for k-tile st: 8 matmuls + evict (ones col = 1)."""
                ensure_xv(st // 2)
                blk = xv_blocks[st // 2]
                off = (st % 2) * 128
                ps = ps_mm.tile([128, 512], F32, tag="ps")
                for t in range(dt_n):
                    nc.tensor.matmul(
                        ps,
                        blk[:, t, off:off + 128],
                        wv_sb[:, t, :],
                        start=(t == 0),
                        stop=(t == dt_n - 1),
                    )
                    yield
                nc.vector.tensor_copy(
                    vn_sb[:, st, :, 0:dk],
                    ps.rearrange("p (h d) -> p h d", d=dk),
                )
                yield

            atn_tiles = {}  # (qc, pr) -> atn_pair tile (normalized, bf16)
            norm_stash = {}  # (qc, pr) -> (stage_A, stage_B, rec_dram)
            prev_pair = [None]  # last finalized (qc, pr) not yet normalized

            def emit_norm(qc, pr):
                """Normalize pair (qc, pr): broadcast 1/den and multiply the
                f32 stage into the bf16 atn tile (head B via DMA shift)."""
                stage_A, stage_B, rd = norm_stash.pop((qc, pr))
                atn_pair = atn_pool.tile([128, 512], BF16, tag="atn")
                for h, st_t in ((0, stage_A), (1, stage_B)):
                    bc = small.tile([64, 512], F32, tag="bc")
                    row = rd[h:h + 1, :]
                    bcast_src = bass.AP(
                        tensor=row.tensor,
                        offset=row.offset,
                        ap=[[0, 64]] + list(row.ap[1:]),
                    )
                    nc.sync.dma_start(out=bc, in_=bcast_src)
                    if h == 0:
                        nc.gpsimd.tensor_mul(
                            atn_pair[0:64, :], st_t[0:64, :], bc)
                    else:
                        btmp = small.tile([64, 512], BF16, tag="btmp")
                        nc.gpsimd.tensor_mul(btmp, st_t[0:64, :], bc)
                        nc.sync.dma_start(out=atn_pair[64:128, :], in_=btmp)
                atn_tiles[(qc, pr)] = atn_pair

            def outproj_gen(qcc, qt_i, ecc):
                esl = slice(ecc * 512, (ecc + 1) * 512)
                q0 = qcc * 4 + qt_i
                o_ps = ps_mm.tile([128, 512], F32, tag="ps")
                for p in range(pairs):
                    nc.tensor.matmul(
                        o_ps,
                        atn_tiles[(qcc, p)][:, qt_i * 128:(qt_i + 1) * 128],
                        wo_sb[:, p, esl],
                        start=(p == 0),
                        stop=(p == pairs - 1),
                    )
                    yield
                o_sb = outsb_pool.tile([128, 512], F32, tag="o")
                nc.vector.tensor_copy(o_sb, o_ps)
                nc.sync.dma_start(
                    out=out.ap()[q0 * 128:(q0 + 1) * 128, esl], in_=o_sb)
                yield

            class FillerQueue:
                def __init__(self):
                    self.tasks = []  # (gen, deadline_step or None)

                def add(self, gen, deadline=None):
                    self.tasks.append((gen, deadline))

                def pump(self, n):
                    while n > 0 and self.tasks:
                        try:
                            next(self.tasks[0][0])
                            n -= 1
                        except StopIteration:
                            self.tasks.pop(0)

                def fence(self, cur):
                    while self.tasks and any(
                        dl is not None and dl <= cur for _, dl in self.tasks
                    ):
                        self.pump(1)

                def drain(self):
                    while self.tasks:
                        self.pump(1000)

            fill = FillerQueue()

            # ---- stage A: just enough for the first scores ----
            # KT and QT chains interleaved per d-tile, paced by the
            # interleaved startup DMAs (two concurrent PSUM accumulators)
            psK = ps_mm.tile([128, 512], F32, tag="ps")
            psQ = ps_mm.tile([128, 512], F32, tag="ps")
            for t in range(dt_n):
                nc.tensor.matmul(
                    psK, wk_sb[:, t, 0:128], xk_sb[:, 0, t, :],
                    start=(t == 0), stop=(t == dt_n - 1))
                nc.tensor.matmul(
                    psQ, wq_sb[:, t, 0:128], xq_blocks[0][:, t, :],
                    start=(t == 0), stop=(t == dt_n - 1))
            nc.vector.tensor_scalar(
                kt_sb[:, 0, 0:512], psK, bkc_sb[:, 0:1], None, ALU.add)
            nc.vector.tensor_scalar(
                qt_sb[:, 0, 0:512], psQ, bqc_sb[:, 0:1], None, ALU.add)

            # stage-A remainder as deadline fillers (all within qc0).
            # AV(kk) is emitted at unit kk+4 during the first pair, so V(st)
            # deadlines get a 2-unit margin past st.
            def vdl(st):
                return step(0, 0, min(st + 2, st_n - 1))

            fill.add(v_gen(0), deadline=step(0, 0, 1))
            fill.add(v_gen(1), deadline=step(0, 0, 2))
            fill.add(v_gen(2), deadline=vdl(2))
            for c in range(1, qc_n):
                fill.add(qk_gen(wk_sb, bkc_sb, xk_sb[:, c, :, :], kt_sb, 0, c),
                         deadline=step(0, 0, 4 * c - 1))
                fill.add(v_gen(2 * c + 1), deadline=vdl(2 * c + 1))
                fill.add(v_gen(2 * c + 2), deadline=vdl(2 * c + 2))
            for st in range(9, st_n):
                fill.add(v_gen(st), deadline=vdl(st))
            # later pairs' KT/QT (kt/qt needed when unit (0, p) starts)
            for p in range(1, pairs):
                fill.add(qk_gen(wq_sb, bqc_sb, xq_blocks[0], qt_sb, p, 0),
                         deadline=step(0, p, 0))
                for c in range(qc_n):
                    fill.add(
                        qk_gen(wk_sb, bkc_sb, xk_sb[:, c, :, :], kt_sb, p, c),
                        deadline=step(0, p, max(4 * c - 1, 0)))

            # ---- AV deque: entries emit the AV matmuls for one unit ----
            avq = []  # (emit_fn, finalize_fn or None)

            def drain_avq(target):
                while len(avq) > target:
                    em, fin = avq.pop(0)
                    em()
                    if fin is not None:
                        fin()

            # ---- main loop over q-chunks and pairs ----
            for qc in range(qc_n):
                qsl = slice(qc * 512, (qc + 1) * 512)
                last = qc == qc_n - 1

                # prefetch xq block qc+1 and enqueue its QT chains
                if qc + 1 < qc_n:
                    blkq = xqin.tile([128, dt_n, 512], BF16, tag="xq")
                    nc.sync.dma_start(
                        out=blkq,
                        in_=xq_src[:, :, (qc + 1) * 512:(qc + 2) * 512])
                    xq_blocks[qc + 1] = blkq
                    for pp in range(pairs):
                        # spread through this chunk's pair 0 (which has ACT
                        # slack) instead of piling into pair 1
                        # FIFO pumping drains these early; the deadline is a
                        # late backstop only (bursts at fences stall ACT)
                        dl = (step(qc, 2, 2 + 4 * pp) if qc > 0
                              else step(qc + 1, pp, 0))
                        fill.add(
                            qk_gen(wq_sb, bqc_sb, blkq, qt_sb, pp, qc + 1),
                            deadline=dl)

                for pr in range(pairs):
                    at_A = ps_at.tile([65, 512], F32, tag="at")
                    at_B = ps_at.tile([65, 512], F32, tag="at")

                    def make_emit(at_A, at_B, pr, kk, e_sb):
                        def em():
                            nc.tensor.matmul(
                                at_A,
                                vn_sb[:, kk, 2 * pr, :],
                                e_sb[:, 0:512],
                                start=(kk == 0),
                                stop=(kk == st_n - 1),
                            )
                            nc.tensor.matmul(
                                at_B,
                                vn_sb[:, kk, 2 * pr + 1, :],
                                e_sb[:, 512:1024],
                                start=(kk == 0),
                                stop=(kk == st_n - 1),
                            )
                        return em

                    def make_finalize(at_A, at_B, qc, pr):
                        def fin():
                            if (qc, pr) == (qc_n - 1, pairs - 1):
                                # last pair: the tail normalizes straight
                                # from the PSUM accumulators (no stage
                                # copies; the banks die with the program)
                                norm_stash[(qc, pr)] = (at_A, at_B, None)
                                if prev_pair[0] is not None:
                                    emit_norm(*prev_pair[0])
                                prev_pair[0] = (qc, pr)
                                return
                            # stage the accumulators (frees both PSUM banks
                            # after one DVE copy each)
                            stage_A = stage_pool.tile([65, 512], F32, tag="st")
                            stage_B = stage_pool.tile([65, 512], F32, tag="st")
                            nc.vector.tensor_copy(stage_A, at_A)
                            nc.vector.tensor_copy(stage_B, at_B)
                            if True:
                                # scatter the [1,512] denominator rows across
                                # 128 partitions so the DVE reciprocal runs on
                                # a free-size-4 AP (~0.15us instead of 3.3us),
                                # then gather back to DRAM for the bcast
                                rsc = small.tile([128, 8], F32, tag="rsc")
                                nc.sync.dma_start(
                                    out=rsc[:, 0:4], in_=stage_A[64:65, :])
                                nc.sync.dma_start(
                                    out=rsc[:, 4:8], in_=stage_B[64:65, :])
                                rcp = small.tile([128, 8], F32, tag="rcp")
                                nc.vector.reciprocal(out=rcp, in_=rsc)
                                rd = dramb.tile([2, 512], F32, tag="rd")
                                nc.sync.dma_start(
                                    out=rd[0:1, :], in_=rcp[:, 0:4])
                                nc.sync.dma_start(
                                    out=rd[1:2, :], in_=rcp[:, 4:8])
                                norm_stash[(qc, pr)] = (stage_A, stage_B, rd)
                            # normalize the previous pair (its rec chain has
                            # had a full pair of lead time)
                            if prev_pair[0] is not None:
                                emit_norm(*prev_pair[0])
                                # when crossing into chunk qc, the previous
                                # chunk becomes fully normalized -> queue its
                                # out-projections
                                pq, ppr = prev_pair[0]
                                if ppr == pairs - 1:
                                    # first deadline leaves time for the
                                    # norm chain of (pq, 3) to resolve
                                    dls = [(1, 13), (1, 15), (2, 3), (2, 7),
                                           (2, 11), (2, 15), (3, 3), (3, 7)]
                                    for sq in range(4 * ec_n):
                                        fill.add(
                                            outproj_gen(
                                                pq, sq // ec_n, sq % ec_n),
                                            deadline=step(pq + 1, *dls[sq]))
                            prev_pair[0] = (qc, pr)
                        return fin

                    for kk in range(st_n):
                        fill.fence(step(qc, pr, kk))
                        sc_ps = ps_sc.tile([128, 1024], F32, tag="sc")
                        ksl = slice(kk * 128, (kk + 1) * 128)
                        nc.tensor.matmul(
                            sc_ps[:, 0:512],
                            kt_sb[0:64, pr, ksl],
                            qt_sb[0:64, pr, qsl],
                            start=True,
                            stop=True,
                        )
                        nc.tensor.matmul(
                            sc_ps[:, 512:1024],
                            kt_sb[64:128, pr, ksl],
                            qt_sb[64:128, pr, qsl],
                            start=True,
                            stop=True,
                        )
                        exp_sb = expst_pool.tile([128, 1024], BF16, tag="e")
                        nc.scalar.activation(
                            exp_sb, sc_ps,
                            mybir.ActivationFunctionType.Exp,
                            scale=1.0 / np.sqrt(dk),
                        )
                        fin = (make_finalize(at_A, at_B, qc, pr)
                               if kk == st_n - 1 else None)
                        avq.append(
                            (make_emit(at_A, at_B, pr, kk, exp_sb), fin))
                        if qc == 0 and pr == 0:
                            depth = 4
                        elif qc == 0 and pr == 1:
                            depth = 3
                        else:
                            depth = 2
                        drain_avq(depth)
                        # steady nibble-sized pumping: bursts create ACT
                        # bubbles via the sc ping-pong
                        fill.pump(2 if (qc == 0 or (kk & 1)) else 1)

            # ---- tail ----
            fill.drain()
            drain_avq(0)  # AV(14), AV(15) + finalize of the last pair
            # Last-pair normalization, no DMA hops: DVE reciprocals straight
            # off the PSUM denominator rows (emitted first so they lead the
            # DVE queue), then -- AFTER the warm reserve matmuls are queued
            # on the PE -- an outer-product broadcast into free ps_sc banks
            # and DVE multiplies into per-head tiles.  The p3 matmuls are
            # split per head, so no partition-shift DMA anywhere.
            lq = qc_n - 1
            at_lA, at_lB, _ = norm_stash.pop((lq, pairs - 1))
            # 1/den = exp(-ln(den)) on the (now idle) ACT engine -- Ln and
            # Exp share an activation table, so this costs ~1.5us with no
            # DVE reciprocal and no DMA hops.
            ln_A = small.tile([1, 512], F32, tag="rbf")
            ln_B = small.tile([1, 512], F32, tag="rbf")
            nc.scalar.activation(
                ln_A, at_lA[64:65, :], mybir.ActivationFunctionType.Ln)
            nc.scalar.activation(
                ln_B, at_lB[64:65, :], mybir.ActivationFunctionType.Ln)
            rec_bA = small.tile([1, 512], BF16, tag="rbb")
            rec_bB = small.tile([1, 512], BF16, tag="rbb")
            nc.scalar.activation(
                rec_bA, ln_A, mybir.ActivationFunctionType.Exp, scale=-1.0)
            nc.scalar.activation(
                rec_bB, ln_B, mybir.ActivationFunctionType.Exp, scale=-1.0)
            # SBUF copies of the accumulators (the multiply may read only
            # one PSUM operand -- the bcast stays in PSUM)
            st_lA = stage_pool.tile([65, 512], F32, tag="st")
            st_lB = stage_pool.tile([65, 512], F32, tag="st")
            nc.vector.tensor_copy(st_lA[0:64, :], at_lA[0:64, :])
            nc.vector.tensor_copy(st_lB[0:64, :], at_lB[0:64, :])

            bc_ps = ps_sc.tile([128, 1024], F32, tag="sc")
            nc.tensor.matmul(
                bc_ps[0:64, 0:512], ones_sb[0:1, 0:64], rec_bA,
                start=True, stop=True)
            nc.tensor.matmul(
                bc_ps[0:64, 512:1024], ones_sb[0:1, 0:64], rec_bB,
                start=True, stop=True)
            atnA = small.tile([64, 512], BF16, tag="tA")
            atnB = small.tile([64, 512], BF16, tag="tB")
            nc.vector.tensor_mul(atnA, st_lA[0:64, :], bc_ps[0:64, 0:512])
            nc.vector.tensor_mul(atnB, st_lB[0:64, :], bc_ps[0:64, 512:1024])

            def tail_chain(o_ps, qt_i, esl, start):
                """p0..p2 accumulation for a last-chunk out-proj chain."""
                for p in range(pairs - 1):
                    nc.tensor.matmul(
                        o_ps,
                        atn_tiles[(lq, p)][:, qt_i * 128:(qt_i + 1) * 128],
                        wo_sb[:, p, esl],
                        start=(p == 0 and start),
                        stop=False,
                    )

            def tail_p3(o_ps, qt_i, ecc, esl):
                """Per-head p3 matmuls + eviction for a last-chunk chain."""
                qsl2 = slice(qt_i * 128, (qt_i + 1) * 128)
                nc.tensor.matmul(
                    o_ps, atnA[:, qsl2], wo_sb[0:64, pairs - 1, esl],
                    start=False, stop=False)
                nc.tensor.matmul(
                    o_ps, atnB[:, qsl2], wo_b3[:, esl],
                    start=False, stop=True)
                q0 = lq * 4 + qt_i
                o_sb = outsb_pool.tile([128, 512], F32, tag="o")
                nc.vector.tensor_copy(o_sb, o_ps)
                nc.sync.dma_start(
                    out=out.ap()[q0 * 128:(q0 + 1) * 128, esl], in_=o_sb)

            held = []
            for sq in range(2):
                qt_i, ecc = sq // ec_n, sq % ec_n
                esl = slice(ecc * 512, (ecc + 1) * 512)
                o_ps = ps_mm.tile([128, 512], F32, tag="ps")
                tail_chain(o_ps, qt_i, esl, start=True)
                held.append((o_ps, qt_i, ecc, esl))
            for o_ps, qt_i, ecc, esl in held:
                tail_p3(o_ps, qt_i, ecc, esl)
            for sq in range(2, 4 * ec_n):
                qt_i, ecc = sq // ec_n, sq % ec_n
                esl = slice(ecc * 512, (ecc + 1) * 512)
                o_ps = ps_mm.tile([128, 512], F32, tag="ps")
                tail_chain(o_ps, qt_i, esl, start=True)
                tail_p3(o_ps, qt_i, ecc, esl)

    nc.compile()
    return nc


_PROGRAM_CACHE = {}


def _get_program(key):
    if key not in _PROGRAM_CACHE:
        _PROGRAM_CACHE[key] = build_program(*key)
    return _PROGRAM_CACHE[key]


def kernel(queries, keys, values, Wq, bq, Wk, bk, Wv, bv, Wo, bo):
    global LAST_EXEC_TIME_NS
    bf16 = ml_dtypes.bfloat16

    nc = _get_program((S, D, HC, D))

    xT = {}
    for name, arr in (("q", queries), ("k", keys), ("v", values)):
        xT[name] = [
            np.ascontiguousarray(np.asarray(arr[b]).T).astype(bf16)
            for b in range(B)
        ]
    Wq, Wk, Wv, Wo = (np.asarray(w) for w in (Wq, Wk, Wv, Wo))
    bqv, bkv, bvv = (np.asarray(v) for v in (bq, bk, bv))

    in_maps = []
    for c in range(N_CORES):
        b, g = c // 2, c % 2
        csl = slice(g * DPC, (g + 1) * DPC)
        in_maps.append(
            {
                "xqT": xT["q"][b],
                "xkT": xT["k"][b],
                "xvT": xT["v"][b],
                "wq": np.ascontiguousarray(Wq[:, csl]).astype(bf16),
                "wk": np.ascontiguousarray(Wk[:, csl]).astype(bf16),
                "wv": np.ascontiguousarray(Wv[:, csl]).astype(bf16),
                "wo": np.ascontiguousarray(Wo[csl, :]).astype(bf16),
                "bq": np.ascontiguousarray(bqv[csl]).astype(np.float32),
                "bk": np.ascontiguousarray(bkv[csl]).astype(np.float32),
            }
        )

    trace = os.environ.get("KERNEL_TRACE", "0") == "1"
    res = run_bass_kernel_spmd(nc, in_maps, list(range(N_CORES)), trace=trace)
    LAST_EXEC_TIME_NS = res.exec_time_ns

    # bv's contribution commutes through softmax-normalized attention:
    # each head's output gains +bv_h, so the final output gains bv @ Wo.
    bo = np.asarray(bo, dtype=np.float32) + bvv.astype(np.float32) @ Wo.astype(np.float32)
    out = np.empty((B, S, D), dtype=np.float32)
    for b in range(B):
        out[b] = res.results[2 * b]["out"] + res.results[2 * b + 1]["out"] + bo
    return out


if __name__ == "__main__":
    t0 = time.time()
    nc = _get_program((S, D, HC, D))
    print(f"build+compile: {time.time() - t0:.1f}s")
